# revision 1
# baseline (speedup 1.0000x reference)
"""BNN MNIST MLP on 8 Trainium2 NeuronCores — pure data parallel.

Model (inference): x[B,784] -> relu(x @ sign(W1)) -> BN1 -> sign ->
@ sign(W2) relu BN2 sign -> @ sign(W3) -> softmax.

Key transformations:
  * BN(relu(h)) >= 0  <=>  h >= t  (per-feature threshold t, since BN scale>0),
    so each binarize step is one ScalarE Sign(h - t) op straight from PSUM.
  * Layer-1 needs fp32-class precision (sign margins ~2.5e-5): x is split on
    host into fp16 hi + lo halves (same total bytes as fp32); both halves are
    stacked into one [1568, B] feature-major tensor and the matmul contracts
    over all 1568 rows against [sign(W1); sign(W1)] — fp16 runs at 1 PE
    cycle/row vs 4 for native fp32, and PSUM accumulates in fp32.
  * x ships pre-transposed (feature-major) per core so the contraction dim
    lands on SBUF partitions with line-rate contiguous DMA; chunks are 128
    partitions wide (full DMA port utilization) and alternate between the
    Sync and Scalar HWDGE rings, prefetched four slabs ahead.
  * The hidden width (50) uses only half the PE array columns, so the two
    512-row groups of each slab run CONCURRENTLY via column tiling
    (tile_position (0,0) / (0,64)) — halving layer-1 streaming time.
  * The slab loop is software-pipelined so the PE instruction stream never
    waits on the ScalarE sign ops: L1(p) is emitted before L2(p-1) and
    L3(p-2); the final slabs de-lag so their dependent stages run during
    the last load window instead of stacking after the last L1 matmul.
  * Layer 3 is fused with the output transpose: its stationary operand is a
    stride-8 batch pick of s2, so each matmul emits batch-major logits
    directly into PSUM (partition q holds rows 8q..8q+7 -> 320 B contiguous
    per partition on the store) — no PSUM->SBUF logit copy and no separate
    PE transpose pass; softmax runs straight on the PSUM tile.
"""
import numpy as np

import concourse.mybir as mybir
from concourse import bacc
from concourse.tile import TileContext
from concourse.bass_utils import run_bass_kernel_spmd

F32 = mybir.dt.float32
F16 = mybir.dt.float16

B = 65536
NCORES = 8
PER = B // NCORES          # 8192 rows per core
SLAB = 1024                # rows per DMA slab
NSLAB = PER // SLAB        # 8
GRP = 512                  # rows per PSUM group (one matmul N)
NGRP = SLAB // GRP         # 2
DSL = 2048                 # rows per transpose/store block (2 slabs)
T = NSLAB * NGRP           # 16 pipeline ticks
K = 784
K2 = 2 * K                 # hi+lo stacked contraction length (1568)
KC = 128                   # contraction chunk (full partition width)
NKC = (K2 + KC - 1) // KC  # 13 chunks: 12 x 128 + 1 x 32
NCLS = 10
NHID = 50
RSTR = DSL // 128          # 16 rows per partition in the output tile

EPS = 1e-3

_CACHE = {}


def _build(prefetch=4, xbufs=5):
    nc = bacc.Bacc("TRN2", target_bir_lowering=False, debug=False,
                   num_devices=NCORES)

    xcat = nc.dram_tensor("xcat", [K2, PER], F16, kind="ExternalInput").ap()
    # all fp16 consts packed in one blob: w1 chunks at cols [50c, 50c+50),
    # w2 at [650, 700), w3 at [700, 710)
    cb16 = nc.dram_tensor("cb16", [128, NHID * NKC + NHID + NCLS], F16,
                          kind="ExternalInput").ap()
    # fp32 consts: col 0 = -T1, col 1 = -T2 (both replicated at partition
    # offset 64 for the column-tiled pair), cols [2, 12) = identity (rows 0-9)
    cb32 = nc.dram_tensor("cb32", [128, 12], F32, kind="ExternalInput").ap()
    out = nc.dram_tensor("out", [PER, NCLS], F32, kind="ExternalOutput").ap()

    kc = [min(KC, K2 - c * KC) for c in range(NKC)]

    with TileContext(nc) as tc:
        with (
            tc.tile_pool(name="consts", bufs=1) as cpool,
            tc.tile_pool(name="xin", bufs=xbufs) as xpool,
            tc.tile_pool(name="mid", bufs=3) as mpool,
            tc.tile_pool(name="fin", bufs=2) as fpool,
            tc.tile_pool(name="psA", bufs=2, space="PSUM") as psA,
            tc.tile_pool(name="psB", bufs=2, space="PSUM") as psB,
        ):
            cb16t = cpool.tile([128, NHID * NKC + NHID + NCLS], F16, tag="cb16")
            nc.sync.dma_start(cb16t[:], cb16[:, :])
            cb32t = cpool.tile([128, 12], F32, tag="cb32")
            nc.scalar.dma_start(cb32t[:], cb32[:, :])
            w1t = [cb16t[0:kc[c], c * NHID:(c + 1) * NHID] for c in range(NKC)]
            w2t = cb16t[0:NHID, NKC * NHID:NKC * NHID + NHID]
            w3t = cb16t[0:NHID, NKC * NHID + NHID:NKC * NHID + NHID + NCLS]
            w2t64 = cb16t[64:64 + NHID, NKC * NHID:NKC * NHID + NHID]
            w3t64 = cb16t[64:64 + NHID,
                          NKC * NHID + NHID:NKC * NHID + NHID + NCLS]
            nt1t = cb32t[0:64 + NHID, 0:1]
            nt2t = cb32t[0:64 + NHID, 1:2]
            idt = cb32t[0:NCLS, 2:12]

            xt = {}
            s1t = {}
            s2t = {}
            s2v = {}

            def emit_loads(s):
                b0 = s * SLAB
                xt[s] = []
                for c in range(NKC):
                    t_ = xpool.tile([kc[c], SLAB], F16, tag=f"x_{c}",
                                    name=f"x_{s}_{c}")
                    eng = nc.sync if c % 2 == 0 else nc.scalar
                    eng.dma_start(t_[:], xcat[c * KC:c * KC + kc[c], b0:b0 + SLAB])
                    xt[s].append(t_)

            def stageA(p):
                # one pair-tick = one slab = 2 groups of 512 rows, run
                # CONCURRENTLY on the PE via column tiling: group 0 on array
                # columns 0-63 (out partitions 0-49), group 1 on columns
                # 64-127 (out partitions 64-113). Halves L1 streaming time.
                s = p
                ps1 = psA.tile([128, GRP], F32, tag="ps1")
                for c in range(NKC):
                    nc.tensor.matmul(ps1[0:NHID, :], w1t[c],
                                     xt[s][c][:, 0:GRP],
                                     start=(c == 0), stop=(c == NKC - 1),
                                     skip_group_check=True)
                    nc.tensor.matmul(ps1[64:64 + NHID, :], w1t[c],
                                     xt[s][c][:, GRP:2 * GRP],
                                     start=(c == 0), stop=(c == NKC - 1),
                                     skip_group_check=True)
                s1 = mpool.tile([64 + NHID, GRP], F16, tag="s1", name=f"s1_{p}")
                nc.scalar.sign(s1[:], ps1[0:64 + NHID, :], bias=nt1t)
                s1t[p] = (s1[0:NHID, :], s1[64:64 + NHID, :])

            def stageB(p):
                ps2 = psA.tile([128, GRP], F32, tag="ps2")
                sa, sb = s1t[p]
                nc.tensor.matmul(ps2[0:NHID, :], w2t, sa,
                                 start=True, stop=True, skip_group_check=True)
                nc.tensor.matmul(ps2[64:64 + NHID, :], w2t64, sb,
                                 start=True, stop=True, skip_group_check=True)
                s2 = mpool.tile([64 + NHID, GRP], F16, tag="s2", name=f"s2_{p}")
                nc.scalar.sign(s2[:], ps2[0:64 + NHID, :], bias=nt2t)
                s2t[p] = (s2[0:NHID, :], s2[64:64 + NHID, :])
                v = s2[:].rearrange("q (j r) -> q j r", r=8)
                s2v[p] = (v[0:NHID, :, :], v[64:64 + NHID, :, :])

            def stageCD(p):
                # Layer 3 fused with the output transpose: the stationary
                # operand is a stride-8 batch pick of s2, so out partition q
                # holds batch rows {8q + r} of the slab -> 320 B contiguous
                # per partition on the store, no PSUM->SBUF copy and no PE
                # transpose pass.
                ps4 = psB.tile([128, 8 * NCLS], F32, tag="ps4", name=f"ps4_{p}")
                s2a3, s2b3 = s2v[p]
                for r in range(8):
                    nc.tensor.matmul(ps4[0:64, r * NCLS:(r + 1) * NCLS],
                                     s2a3[:, :, r], w3t,
                                     start=True, stop=True,
                                     skip_group_check=True)
                    nc.tensor.matmul(ps4[64:128, r * NCLS:(r + 1) * NCLS],
                                     s2b3[:, :, r], w3t64,
                                     start=True, stop=True,
                                     skip_group_check=True)
                eo = fpool.tile([128, 8 * NCLS], F32, tag="eo", name=f"eo_{p}")
                nc.scalar.activation(eo[:], ps4[:],
                                     mybir.ActivationFunctionType.Exp)
                sm = fpool.tile([128, 8], F32, tag="sm", name=f"sm_{p}")
                eov = eo[:].rearrange("q (r c) -> q r c", c=NCLS)
                nc.vector.tensor_reduce(sm[:], eov, axis=mybir.AxisListType.X,
                                        op=mybir.AluOpType.add)
                rv = fpool.tile([128, 8], F32, tag="rv", name=f"rv_{p}")
                nc.vector.reciprocal(rv[:], sm[:])
                ot = fpool.tile([128, 8 * NCLS], F32, tag="ot", name=f"ot_{p}")
                otv = ot[:].rearrange("q (r c) -> q r c", c=NCLS)
                rvb = rv[:].unsqueeze(-1).broadcast_to([128, 8, NCLS])
                nc.vector.tensor_mul(otv, eov, rvb)
                b0 = p * SLAB
                dst = out[b0:b0 + SLAB, :].rearrange("(q r) f -> q (r f)", q=128)
                nc.sync.dma_start(dst, ot[:])

            # steady state keeps a 1/2-slab lag so the PE FIFO never waits
            # on ScalarE; the final slabs de-lag so their dependent stages run
            # during the last load window instead of stacking after A(7)
            for s in range(min(prefetch, NSLAB)):
                emit_loads(s)
            for p in range(NSLAB - 2):
                stageA(p)
                if p + prefetch < NSLAB:
                    emit_loads(p + prefetch)
                if p >= 1:
                    stageB(p - 1)
                if p >= 2:
                    stageCD(p - 2)
            stageB(NSLAB - 3)      # B(5)
            stageA(NSLAB - 2)      # A(6) first: streams while slab 6 lands
            stageCD(NSLAB - 4)     # CD(4)
            stageB(NSLAB - 2)      # B(6)
            stageA(NSLAB - 1)      # A(7) queues right behind, streams as
            stageCD(NSLAB - 3)     # CD(5)   slab 7 lands; CD(5)/CD(6) fill
            stageCD(NSLAB - 2)     # CD(6)   the DMA-wait slack behind it
            stageB(NSLAB - 1)      # B(7)
            stageCD(NSLAB - 1)     # CD(7)

    nc.compile()
    return nc


def _prep_host(inputs, W1, W2, W3, g1, b1, m1, v1, g2, b2, m2, v2):
    x = np.ascontiguousarray(inputs.reshape(B, K).astype(np.float32, copy=False))
    xhi = x.astype(np.float16)
    xlo = (x - xhi.astype(np.float32)).astype(np.float16)

    w1b = np.where(W1 >= 0, 1.0, -1.0).astype(np.float16)
    w2b = np.where(W2 >= 0, 1.0, -1.0).astype(np.float16)
    w3b = np.where(W3 >= 0, 1.0, -1.0).astype(np.float16)

    a1 = g1.astype(np.float64) / np.sqrt(v1.astype(np.float64) + EPS)
    c1 = b1.astype(np.float64) - a1 * m1.astype(np.float64)
    t1 = -c1 / a1
    T1 = np.where(t1 > 0, t1, -1e30).astype(np.float32)
    a2 = g2.astype(np.float64) / np.sqrt(v2.astype(np.float64) + EPS)
    c2 = b2.astype(np.float64) - a2 * m2.astype(np.float64)
    t2 = -c2 / a2
    T2 = np.where(t2 > 0, t2, -1e30).astype(np.float32)

    w1cat = np.vstack([w1b, w1b])
    cb16 = np.zeros((128, NHID * NKC + NHID + NCLS), dtype=np.float16)
    for c in range(NKC):
        n = min(KC, K2 - c * KC)
        cb16[:n, c * NHID:(c + 1) * NHID] = w1cat[c * KC:c * KC + n]
    cb16[:NHID, NKC * NHID:NKC * NHID + NHID] = w2b
    cb16[:NHID, NKC * NHID + NHID:] = w3b
    cb16[64:64 + NHID, NKC * NHID:NKC * NHID + NHID] = w2b
    cb16[64:64 + NHID, NKC * NHID + NHID:] = w3b
    cb32 = np.zeros((128, 12), dtype=np.float32)
    cb32[:NHID, 0] = -T1
    cb32[64:64 + NHID, 0] = -T1
    cb32[:NHID, 1] = -T2
    cb32[64:64 + NHID, 1] = -T2
    cb32[:NCLS, 2:12] = np.eye(NCLS, dtype=np.float32)
    shared = {"cb16": cb16, "cb32": cb32}
    in_maps = []
    for c in range(NCORES):
        sl = slice(c * PER, (c + 1) * PER)
        m = dict(shared)
        xc = np.empty((K2, PER), dtype=np.float16)
        xc[:K] = xhi[sl].T
        xc[K:] = xlo[sl].T
        m["xcat"] = xc
        in_maps.append(m)
    return in_maps


def kernel(**inputs):
    if "nc" not in _CACHE:
        _CACHE["nc"] = _build()
    nc = _CACHE["nc"]
    inputs = {k: np.asarray(v) for k, v in inputs.items()}
    in_maps = _prep_host(**inputs)
    res = run_bass_kernel_spmd(nc, in_maps, core_ids=list(range(NCORES)))
    return np.concatenate([r["out"] for r in res.results], axis=0)



# revision 6
# speedup vs baseline: 1.0813x; 1.0813x over previous
"""BNN MNIST MLP on 8 Trainium2 NeuronCores — pure data parallel, v2.

Model (inference): x[B,784] -> relu(x @ sign(W1)) -> BN1 -> sign ->
@ sign(W2) relu BN2 sign -> @ sign(W3) -> softmax.

v2 design (vs v1 baseline at ~118us):
  * 25% less HBM traffic: x ships as fp16 hi (2B) + fp8-e5m2 residual
    (1B, scaled by 2^12; matching stationary sign(W1)*2^-12 is exact in
    e5m2).  Validated rel_err 8.5e-3 vs the 2e-2 gate on the harness
    inputs.  19.3MB/core vs 25.7MB.
  * Fully SBUF-resident input (168KB/partition < 208KB usable): all 112
    chunk loads are issued up-front on three otherwise-idle DMA rings
    (sync/vector/gpsimd) with no buffer-reuse gating, so DMA streams at
    wire speed the whole run instead of sagging on compute backpressure.
  * Compute engines issue no loads: ACT does sign/copy only; per-slab
    dma_start sequencer cost (~600ns each) moves off the critical path.
  * Layer 3 emits class-major integer logits ([10, B], exact in fp16);
    softmax and the final transpose run on host numpy.  This deletes the
    16-tiny-matmul PE transpose of v1 (~4us/slab of PE time) and the
    vector softmax, and makes stores 1KB-contiguous lines.
  * BN(relu(h)) >= 0  <=>  h >= t per-feature threshold as in v1: each
    binarize is one ScalarE Sign(h - t) straight from PSUM.
  * Column tiling: the two 512-row groups of each slab run concurrently
    on PE array columns 0-63 / 64-127 (confirmed 427ns/pair cadence).
"""
import numpy as np
import ml_dtypes

import concourse.mybir as mybir
from concourse import bacc
from concourse.tile import TileContext
from concourse.bass_utils import run_bass_kernel_spmd

F32 = mybir.dt.float32
F16 = mybir.dt.float16
F8 = mybir.dt.float8e5

B = 65536
NCORES = 8
PER = B // NCORES          # 8192 rows per core
SLAB = 1024                # rows per slab (2 column-tiled groups of 512)
NSLAB = PER // SLAB        # 8
GRP = 512                  # rows per PSUM group (one matmul N)
K = 784
KC = 112                   # contraction chunk rows
NKC = K // KC              # 7 chunks
NCLS = 10
NHID = 50
LOSC = 4096.0              # residual scale 2^12

EPS = 1e-3

_CACHE = {}


def _build():
    nc = bacc.Bacc("TRN2", target_bir_lowering=False, debug=False,
                   num_devices=NCORES)

    xhi = nc.dram_tensor("xhi", [K, PER], F16, kind="ExternalInput").ap()
    xlo = nc.dram_tensor("xlo", [K, PER], F8, kind="ExternalInput").ap()
    # fp16 consts: w1hi chunk c at cols [50c,50c+50) (partitions 0-111),
    # w2 at [350,400), w3 at [400,410) (both at partitions 0-49 AND 64-113)
    cb16 = nc.dram_tensor("cb16", [128, NKC * NHID + NHID + NCLS], F16,
                          kind="ExternalInput").ap()
    # fp8 consts: w1lo chunk c (= sign(W1)*2^-12) at cols [50c, 50c+50)
    cb8 = nc.dram_tensor("cb8", [128, NKC * NHID], F8,
                         kind="ExternalInput").ap()
    # fp32 consts: col 0 = -T1, col 1 = -T2 (replicated at partition 64)
    cb32 = nc.dram_tensor("cb32", [128, 2], F32, kind="ExternalInput").ap()
    out16 = nc.dram_tensor("out16", [NCLS, PER], F16,
                           kind="ExternalOutput").ap()

    with TileContext(nc) as tc:
        with (
            tc.tile_pool(name="consts", bufs=1) as cpool,
            tc.tile_pool(name="xin", bufs=1) as xpool,
            tc.tile_pool(name="mid", bufs=3) as mpool,
            tc.tile_pool(name="fin", bufs=2) as fpool,
            tc.tile_pool(name="psA", bufs=2, space="PSUM") as psA,
            tc.tile_pool(name="psB", bufs=2, space="PSUM") as psB,
            tc.tile_pool(name="psC", bufs=2, space="PSUM") as psC,
        ):
            # consts on the scalar ring (idle at startup; x rings start at 0)
            cb16t = cpool.tile([128, NKC * NHID + NHID + NCLS], F16, tag="cb16")
            nc.scalar.dma_start(cb16t[:], cb16[:, :])
            cb8t = cpool.tile([128, NKC * NHID], F8, tag="cb8")
            nc.scalar.dma_start(cb8t[:], cb8[:, :])
            cb32t = cpool.tile([128, 2], F32, tag="cb32")
            nc.scalar.dma_start(cb32t[:], cb32[:, :])

            w1h = [cb16t[0:KC, c * NHID:(c + 1) * NHID] for c in range(NKC)]
            w1l = [cb8t[0:KC, c * NHID:(c + 1) * NHID] for c in range(NKC)]
            w2t = cb16t[0:NHID, NKC * NHID:NKC * NHID + NHID]
            w2t64 = cb16t[64:64 + NHID, NKC * NHID:NKC * NHID + NHID]
            w3t = cb16t[0:NHID, NKC * NHID + NHID:NKC * NHID + NHID + NCLS]
            w3t64 = cb16t[64:64 + NHID,
                          NKC * NHID + NHID:NKC * NHID + NHID + NCLS]
            nt1t = cb32t[0:64 + NHID, 0:1]
            nt2t = cb32t[0:64 + NHID, 1:2]

            # ---- all x loads up-front, batched multi-chunk DMAs, one ring
            # each: sync = hi chunks 0-3, scalar = hi chunks 4-6, gpsimd =
            # all 7 lo chunks.  24 load DMAs total; no buffer gating.
            HA = 4                     # hi chunks in the sync batch
            HB = NKC - HA              # hi chunks in the scalar batch
            xha = [None] * NSLAB       # [112, HA*1024] fp16
            xhb = [None] * NSLAB       # [112, HB*1024] fp16
            xlt = [None] * NSLAB       # [112, 7*1024] fp8
            for s in range(NSLAB):
                b0 = s * SLAB
                ta = xpool.tile([KC, HA * SLAB], F16, tag=f"xha{s}")
                nc.sync.dma_start(
                    ta[:].rearrange("p (c j) -> p c j", c=HA),
                    xhi[0:HA * KC, b0:b0 + SLAB]
                    .rearrange("(c p) j -> p c j", p=KC))
                xha[s] = ta
                tb = xpool.tile([KC, HB * SLAB], F16, tag=f"xhb{s}")
                nc.scalar.dma_start(
                    tb[:].rearrange("p (c j) -> p c j", c=HB),
                    xhi[HA * KC:NKC * KC, b0:b0 + SLAB]
                    .rearrange("(c p) j -> p c j", p=KC))
                xhb[s] = tb
                tl = xpool.tile([KC, NKC * SLAB], F8, tag=f"xl{s}")
                nc.gpsimd.dma_start(
                    tl[:].rearrange("p (c j) -> p c j", c=NKC),
                    xlo[:, b0:b0 + SLAB]
                    .rearrange("(c p) j -> p c j", p=KC))
                xlt[s] = tl

            s1t = {}
            s2t = {}

            def stageA(s):
                ps1 = psA.tile([128, GRP], F32, tag="ps1")
                for c in range(NKC):
                    st = (c == 0)
                    xin = (xha[s][:, c * SLAB:c * SLAB + SLAB] if c < HA else
                           xhb[s][:, (c - HA) * SLAB:(c - HA + 1) * SLAB])
                    nc.tensor.matmul(ps1[0:NHID, :], w1h[c],
                                     xin[:, 0:GRP],
                                     start=st, stop=False,
                                     skip_group_check=True)
                    nc.tensor.matmul(ps1[64:64 + NHID, :], w1h[c],
                                     xin[:, GRP:2 * GRP],
                                     start=st, stop=False,
                                     skip_group_check=True)
                for c in range(NKC):
                    sp = (c == NKC - 1)
                    xin = xlt[s][:, c * SLAB:(c + 1) * SLAB]
                    nc.tensor.matmul(ps1[0:NHID, :], w1l[c],
                                     xin[:, 0:GRP],
                                     start=False, stop=sp,
                                     skip_group_check=True)
                    nc.tensor.matmul(ps1[64:64 + NHID, :], w1l[c],
                                     xin[:, GRP:2 * GRP],
                                     start=False, stop=sp,
                                     skip_group_check=True)
                s1 = mpool.tile([64 + NHID, GRP], F16, tag="s1",
                                name=f"s1_{s}")
                nc.scalar.sign(s1[:], ps1[0:64 + NHID, :], bias=nt1t)
                s1t[s] = s1

            def stageB(p):
                ps2 = psB.tile([128, GRP], F32, tag="ps2")
                s1 = s1t[p]
                nc.tensor.matmul(ps2[0:NHID, :], w2t, s1[0:NHID, :],
                                 start=True, stop=True, skip_group_check=True)
                nc.tensor.matmul(ps2[64:64 + NHID, :], w2t64,
                                 s1[64:64 + NHID, :],
                                 start=True, stop=True, skip_group_check=True)
                s2 = mpool.tile([64 + NHID, GRP], F16, tag="s2",
                                name=f"s2_{p}")
                nc.scalar.sign(s2[:], ps2[0:64 + NHID, :], bias=nt2t)
                s2t[p] = s2

            def stageC(p):
                ps3 = psC.tile([74, GRP], F32, tag="ps3")
                s2 = s2t[p]
                nc.tensor.matmul(ps3[0:NCLS, :], w3t, s2[0:NHID, :],
                                 start=True, stop=True, skip_group_check=True)
                nc.tensor.matmul(ps3[64:64 + NCLS, :], w3t64,
                                 s2[64:64 + NHID, :],
                                 start=True, stop=True, skip_group_check=True)
                lg = fpool.tile([64 + NCLS, GRP], F16, tag="lg",
                                name=f"lg_{p}")
                nc.vector.tensor_scalar_add(lg[0:NCLS, :], ps3[0:NCLS, :], 0.0)
                nc.vector.tensor_scalar_add(lg[64:64 + NCLS, :],
                                            ps3[64:64 + NCLS, :], 0.0)
                b0 = p * SLAB
                nc.gpsimd.dma_start(out16[:, b0:b0 + GRP], lg[0:NCLS, :])
                nc.gpsimd.dma_start(out16[:, b0 + GRP:b0 + SLAB],
                                    lg[64:64 + NCLS, :])

            # software pipeline: PE never waits on ScalarE signs
            for p in range(NSLAB):
                stageA(p)
                if p >= 1:
                    stageB(p - 1)
                if p >= 2:
                    stageC(p - 2)
            stageB(NSLAB - 1)
            stageC(NSLAB - 2)
            stageC(NSLAB - 1)

    nc.compile()
    return nc


def _prep_host(inputs, W1, W2, W3, g1, b1, m1, v1, g2, b2, m2, v2):
    x = np.ascontiguousarray(inputs.reshape(B, K).astype(np.float32,
                                                         copy=False))
    xhi = x.astype(np.float16)
    xlo = ((x - xhi.astype(np.float32)) * LOSC).astype(ml_dtypes.float8_e5m2)

    w1b = np.where(W1 >= 0, 1.0, -1.0).astype(np.float16)
    w2b = np.where(W2 >= 0, 1.0, -1.0).astype(np.float16)
    w3b = np.where(W3 >= 0, 1.0, -1.0).astype(np.float16)
    w1l = (w1b.astype(np.float32) / LOSC).astype(ml_dtypes.float8_e5m2)

    a1 = g1.astype(np.float64) / np.sqrt(v1.astype(np.float64) + EPS)
    c1 = b1.astype(np.float64) - a1 * m1.astype(np.float64)
    t1 = -c1 / a1
    T1 = np.where(t1 > 0, t1, -1e30).astype(np.float32)
    a2 = g2.astype(np.float64) / np.sqrt(v2.astype(np.float64) + EPS)
    c2 = b2.astype(np.float64) - a2 * m2.astype(np.float64)
    t2 = -c2 / a2
    T2 = np.where(t2 > 0, t2, -1e30).astype(np.float32)

    cb16 = np.zeros((128, NKC * NHID + NHID + NCLS), dtype=np.float16)
    cb8 = np.zeros((128, NKC * NHID), dtype=ml_dtypes.float8_e5m2)
    for c in range(NKC):
        cb16[:KC, c * NHID:(c + 1) * NHID] = w1b[c * KC:(c + 1) * KC]
        cb8[:KC, c * NHID:(c + 1) * NHID] = w1l[c * KC:(c + 1) * KC]
    cb16[:NHID, NKC * NHID:NKC * NHID + NHID] = w2b
    cb16[:NHID, NKC * NHID + NHID:] = w3b
    cb16[64:64 + NHID, NKC * NHID:NKC * NHID + NHID] = w2b
    cb16[64:64 + NHID, NKC * NHID + NHID:] = w3b
    cb32 = np.zeros((128, 2), dtype=np.float32)
    cb32[:NHID, 0] = -T1
    cb32[64:64 + NHID, 0] = -T1
    cb32[:NHID, 1] = -T2
    cb32[64:64 + NHID, 1] = -T2
    shared = {"cb16": cb16, "cb8": cb8, "cb32": cb32}
    in_maps = []
    for c in range(NCORES):
        sl = slice(c * PER, (c + 1) * PER)
        m = dict(shared)
        m["xhi"] = np.ascontiguousarray(xhi[sl].T)
        m["xlo"] = np.ascontiguousarray(xlo[sl].T)
        in_maps.append(m)
    return in_maps


def kernel(**inputs):
    if "nc" not in _CACHE:
        _CACHE["nc"] = _build()
    nc = _CACHE["nc"]
    inputs = {k: np.asarray(v) for k, v in inputs.items()}
    in_maps = _prep_host(**inputs)
    res = run_bass_kernel_spmd(nc, in_maps, core_ids=list(range(NCORES)))
    # class-major integer logits -> softmax on host (fp32, exact match)
    lg = np.concatenate([r["out16"] for r in res.results], axis=1)
    lg = lg.T.astype(np.float32)                      # [B, 10]
    e = np.exp(lg - lg.max(axis=1, keepdims=True))
    return (e / e.sum(axis=1, keepdims=True)).astype(np.float32)


# revision 7
# speedup vs baseline: 1.1506x; 1.0641x over previous
"""BNN MNIST MLP on 8 Trainium2 NeuronCores — pure data parallel, v2c.

Model (inference): x[B,784] -> relu(x @ sign(W1)) -> BN1 -> sign ->
@ sign(W2) relu BN2 sign -> @ sign(W3) -> softmax.

Design (vs the 118us v1 baseline):
  * 25% less HBM traffic: x ships as fp16 hi (2B) + fp8-e5m2 residual
    (1B, scaled 2^12; the matching stationary sign(W1)*2^-12 is exact in
    e5m2).  Validated rel_err 8.45e-3 vs the 2e-2 gate.
  * Input is fully SBUF-resident (~168KB/partition): every load is
    issued up-front with no buffer-reuse gating, so DMA free-runs.
  * Strict engine/ring separation — no compute op ever queues behind a
    blocked dma_start: sync + scalar rings carry only hi chunk loads
    (chunk-granular 2D DMAs, the fast HWDGE path), gpsimd carries the
    7 full-row lo tiles (8KB descriptors) plus the output stores, DVE
    does all elementwise work, PE only matmuls.
  * Binarize via DVE tensor_scalar is_ge -> activations in {1,0}: the
    +-1 encoding is recovered exactly by folding 2s-1 into the next
    layer's threshold (t2' = (t2 + colsum(W2b))/2) and, for the last
    layer, into the host-side softmax (logits = 2*lg' - colsum(W3b)).
    This removes the ScalarE from the pipeline entirely.
  * Layer 3 emits class-major logits ([10, B], half-integers, exact in
    fp16); softmax + transpose run on host.  No PE transpose pass.
  * Column tiling: each slab's two 512-col groups run concurrently on
    PE array columns 0-63 / 64-127.
"""
import numpy as np
import ml_dtypes

import concourse.mybir as mybir
from concourse import bacc
from concourse.tile import TileContext
from concourse.bass_utils import run_bass_kernel_spmd

F32 = mybir.dt.float32
F16 = mybir.dt.float16
F8 = mybir.dt.float8e5
GE = mybir.AluOpType.is_ge

B = 65536
NCORES = 8
PER = B // NCORES          # 8192 rows per core
SLAB = 1024                # rows per slab (2 column-tiled groups of 512)
NSLAB = PER // SLAB        # 8
GRP = 512
K = 784
KC = 112                   # contraction chunk rows
NKC = K // KC              # 7 chunks
NCLS = 10
NHID = 50
LOSC = 4096.0              # residual scale 2^12

EPS = 1e-3

_CACHE = {}


def _build():
    nc = bacc.Bacc("TRN2", target_bir_lowering=False, debug=False,
                   num_devices=NCORES)

    xhi = nc.dram_tensor("xhi", [K, PER], F16, kind="ExternalInput").ap()
    xlo = nc.dram_tensor("xlo", [K, PER], F8, kind="ExternalInput").ap()
    cb16 = nc.dram_tensor("cb16", [128, NKC * NHID + NHID + NCLS], F16,
                          kind="ExternalInput").ap()
    cb8 = nc.dram_tensor("cb8", [128, NKC * NHID], F8,
                         kind="ExternalInput").ap()
    # col 0 = T1, col 1 = T2eff (both replicated at partition offset 64)
    cb32 = nc.dram_tensor("cb32", [128, 2], F32, kind="ExternalInput").ap()
    out16 = nc.dram_tensor("out16", [NCLS, PER], F16,
                           kind="ExternalOutput").ap()

    with TileContext(nc) as tc:
        with (
            tc.tile_pool(name="consts", bufs=1) as cpool,
            tc.tile_pool(name="xin", bufs=1) as xpool,
            tc.tile_pool(name="mid", bufs=3) as mpool,
            tc.tile_pool(name="fin", bufs=2) as fpool,
            tc.tile_pool(name="psA", bufs=2, space="PSUM") as psA,
            tc.tile_pool(name="psB", bufs=2, space="PSUM") as psB,
            tc.tile_pool(name="psC", bufs=2, space="PSUM") as psC,
        ):
            cb16t = cpool.tile([128, NKC * NHID + NHID + NCLS], F16, tag="cb16")
            nc.scalar.dma_start(cb16t[:], cb16[:, :])
            cb8t = cpool.tile([128, NKC * NHID], F8, tag="cb8")
            nc.scalar.dma_start(cb8t[:], cb8[:, :])
            cb32t = cpool.tile([128, 2], F32, tag="cb32")
            nc.scalar.dma_start(cb32t[:], cb32[:, :])

            w1h = [cb16t[0:KC, c * NHID:(c + 1) * NHID] for c in range(NKC)]
            w1l = [cb8t[0:KC, c * NHID:(c + 1) * NHID] for c in range(NKC)]
            w2t = cb16t[0:NHID, NKC * NHID:NKC * NHID + NHID]
            w2t64 = cb16t[64:64 + NHID, NKC * NHID:NKC * NHID + NHID]
            w3t = cb16t[0:NHID, NKC * NHID + NHID:NKC * NHID + NHID + NCLS]
            w3t64 = cb16t[64:64 + NHID,
                          NKC * NHID + NHID:NKC * NHID + NHID + NCLS]
            t1t = cb32t[0:64 + NHID, 0:1]
            t2t = cb32t[0:64 + NHID, 1:2]

            # lo: 7 full-row resident tiles, 8KB-contiguous descriptors,
            # on the gpsimd (SWDGE) ring — all issued at t=0
            lot = []
            for c in range(NKC):
                t_ = xpool.tile([KC, PER], F8, tag=f"xl{c}")
                nc.gpsimd.dma_start(t_[:], xlo[c * KC:(c + 1) * KC, :])
                lot.append(t_)

            # hi: chunk-granular per-slab 2D loads, sync/scalar by parity
            xh = [[None] * NKC for _ in range(NSLAB)]
            for s in range(NSLAB):
                b0 = s * SLAB
                for c in range(NKC):
                    t_ = xpool.tile([KC, SLAB], F16, tag=f"xh{s}_{c}")
                    eng = nc.sync if c % 2 == 0 else nc.scalar
                    eng.dma_start(t_[:], xhi[c * KC:(c + 1) * KC,
                                             b0:b0 + SLAB])
                    xh[s][c] = t_

            s1t = {}
            s2t = {}

            def stageA(s):
                ps1 = psA.tile([128, GRP], F32, tag="ps1")
                for c in range(NKC):
                    st = (c == 0)
                    xin = xh[s][c]
                    nc.tensor.matmul(ps1[0:NHID, :], w1h[c], xin[:, 0:GRP],
                                     start=st, stop=False,
                                     skip_group_check=True)
                    nc.tensor.matmul(ps1[64:64 + NHID, :], w1h[c],
                                     xin[:, GRP:2 * GRP],
                                     start=st, stop=False,
                                     skip_group_check=True)
                b0 = s * SLAB
                for c in range(NKC):
                    sp = (c == NKC - 1)
                    nc.tensor.matmul(ps1[0:NHID, :], w1l[c],
                                     lot[c][:, b0:b0 + GRP],
                                     start=False, stop=sp,
                                     skip_group_check=True)
                    nc.tensor.matmul(ps1[64:64 + NHID, :], w1l[c],
                                     lot[c][:, b0 + GRP:b0 + SLAB],
                                     start=False, stop=sp,
                                     skip_group_check=True)
                s1 = mpool.tile([64 + NHID, GRP], F16, tag="s1",
                                name=f"s1_{s}")
                nc.vector.tensor_scalar(s1[:], ps1[0:64 + NHID, :],
                                        t1t, None, GE)
                s1t[s] = s1

            def stageB(p):
                ps2 = psB.tile([128, GRP], F32, tag="ps2")
                s1 = s1t[p]
                nc.tensor.matmul(ps2[0:NHID, :], w2t, s1[0:NHID, :],
                                 start=True, stop=True, skip_group_check=True)
                nc.tensor.matmul(ps2[64:64 + NHID, :], w2t64,
                                 s1[64:64 + NHID, :],
                                 start=True, stop=True, skip_group_check=True)
                s2 = mpool.tile([64 + NHID, GRP], F16, tag="s2",
                                name=f"s2_{p}")
                nc.vector.tensor_scalar(s2[:], ps2[0:64 + NHID, :],
                                        t2t, None, GE)
                s2t[p] = s2

            def stageC(p):
                ps3 = psC.tile([74, GRP], F32, tag="ps3")
                s2 = s2t[p]
                nc.tensor.matmul(ps3[0:NCLS, :], w3t, s2[0:NHID, :],
                                 start=True, stop=True, skip_group_check=True)
                nc.tensor.matmul(ps3[64:64 + NCLS, :], w3t64,
                                 s2[64:64 + NHID, :],
                                 start=True, stop=True, skip_group_check=True)
                lg = fpool.tile([64 + NCLS, GRP], F16, tag="lg",
                                name=f"lg_{p}")
                nc.vector.tensor_scalar_add(lg[0:NCLS, :], ps3[0:NCLS, :], 0.0)
                nc.vector.tensor_scalar_add(lg[64:64 + NCLS, :],
                                            ps3[64:64 + NCLS, :], 0.0)
                b0 = p * SLAB
                nc.gpsimd.dma_start(out16[:, b0:b0 + GRP], lg[0:NCLS, :])
                nc.gpsimd.dma_start(out16[:, b0 + GRP:b0 + SLAB],
                                    lg[64:64 + NCLS, :])

            for p in range(NSLAB):
                stageA(p)
                if p >= 1:
                    stageB(p - 1)
                if p >= 2:
                    stageC(p - 2)
            stageB(NSLAB - 1)
            stageC(NSLAB - 2)
            stageC(NSLAB - 1)

    nc.compile()
    return nc


def _prep_host(inputs, W1, W2, W3, g1, b1, m1, v1, g2, b2, m2, v2):
    x = np.ascontiguousarray(inputs.reshape(B, K).astype(np.float32,
                                                         copy=False))
    xhi = x.astype(np.float16)
    xlo = ((x - xhi.astype(np.float32)) * LOSC).astype(ml_dtypes.float8_e5m2)

    w1b = np.where(W1 >= 0, 1.0, -1.0).astype(np.float16)
    w2b = np.where(W2 >= 0, 1.0, -1.0).astype(np.float16)
    w3b = np.where(W3 >= 0, 1.0, -1.0).astype(np.float16)
    w1l = (w1b.astype(np.float32) / LOSC).astype(ml_dtypes.float8_e5m2)

    def thresh(g, b, m, v):
        a = g.astype(np.float64) / np.sqrt(v.astype(np.float64) + EPS)
        c = b.astype(np.float64) - a * m.astype(np.float64)
        t = -c / a
        return np.where(t > 0, t, -1e30).astype(np.float32)

    T1 = thresh(g1, b1, m1, v1)
    T2 = thresh(g2, b2, m2, v2)
    c2 = w2b.astype(np.float32).sum(axis=0)
    T2e = ((T2 + c2) / 2).astype(np.float32)

    cb16 = np.zeros((128, NKC * NHID + NHID + NCLS), dtype=np.float16)
    cb8 = np.zeros((128, NKC * NHID), dtype=ml_dtypes.float8_e5m2)
    for c in range(NKC):
        cb16[:KC, c * NHID:(c + 1) * NHID] = w1b[c * KC:(c + 1) * KC]
        cb8[:KC, c * NHID:(c + 1) * NHID] = w1l[c * KC:(c + 1) * KC]
    cb16[:NHID, NKC * NHID:NKC * NHID + NHID] = w2b
    cb16[:NHID, NKC * NHID + NHID:] = w3b
    cb16[64:64 + NHID, NKC * NHID:NKC * NHID + NHID] = w2b
    cb16[64:64 + NHID, NKC * NHID + NHID:] = w3b
    cb32 = np.zeros((128, 2), dtype=np.float32)
    cb32[:NHID, 0] = T1
    cb32[64:64 + NHID, 0] = T1
    cb32[:NHID, 1] = T2e
    cb32[64:64 + NHID, 1] = T2e
    shared = {"cb16": cb16, "cb8": cb8, "cb32": cb32}
    in_maps = []
    for c in range(NCORES):
        sl = slice(c * PER, (c + 1) * PER)
        m = dict(shared)
        m["xhi"] = np.ascontiguousarray(xhi[sl].T)
        m["xlo"] = np.ascontiguousarray(xlo[sl].T)
        in_maps.append(m)
    return in_maps


def kernel(**inputs):
    if "nc" not in _CACHE:
        _CACHE["nc"] = _build()
    nc = _CACHE["nc"]
    inputs = {k: np.asarray(v) for k, v in inputs.items()}
    in_maps = _prep_host(**inputs)
    res = run_bass_kernel_spmd(nc, in_maps, core_ids=list(range(NCORES)))
    # device logits' use {1,0} activations: true logits = 2*lg' - colsum(W3b)
    w3b = np.where(inputs["W3"] >= 0, 1.0, -1.0).astype(np.float32)
    c3 = w3b.sum(axis=0)
    lg = np.concatenate([r["out16"] for r in res.results], axis=1)
    lg = 2.0 * lg.T.astype(np.float32) - c3                  # [B, 10]
    e = np.exp(lg - lg.max(axis=1, keepdims=True))
    return (e / e.sum(axis=1, keepdims=True)).astype(np.float32)


# revision 8
# speedup vs baseline: 1.2400x; 1.0777x over previous
"""BNN MNIST MLP on 8 Trainium2 NeuronCores — pure data parallel, v2d.

Model (inference): x[B,784] -> relu(x @ sign(W1)) -> BN1 -> sign ->
@ sign(W2) relu BN2 sign -> @ sign(W3) -> softmax.

Design (vs the 118us v1 baseline):
  * 25% less HBM traffic: x ships as fp16 hi (2B) + fp8-e5m2 residual
    (1B, scaled 2^12; the matching stationary sign(W1)*2^-12 is exact in
    e5m2).  Validated rel_err 8.45e-3 vs the 2e-2 gate.
  * Input fully SBUF-resident; every load issued up-front, unthrottled.
  * DMA efficiency: per-engine throughput ~ bytes/(45ns + bytes/24GBps),
    so lines must be >=4KB: hi ships as slab-PAIR chunk tiles
    ([128, 2048] fp16 -> 4KB lines) and lo as full-row chunk tiles
    ([128, 8192] fp8 -> 8KB lines).  128-row chunks (6x128+16) keep all
    partitions loaded.
  * lo tiles are spread across all three DMA rings and emitted early so
    the last-consumed bytes are hi of the last slab pair, not lo: kills
    the 18us drain tail of the single-queue version.  ps1 gets 4 PSUM
    banks so early slabs hold accumulations open while lo streams in.
  * Strict ring/engine separation: sync + scalar + gpsimd rings carry
    loads only (plus gpsimd the tiny stores), DVE does all elementwise
    work (is_ge binarize in {1,0} encoding, exact via threshold folding
    t2' = (t2+colsum(W2b))/2 and host-side logits = 2*lg' - colsum(W3b)),
    PE only matmuls.  ScalarE runs nothing per-slab.
  * Layer 3 emits class-major logits ([10, B], half-integers, exact in
    fp16); softmax + transpose on host.  No PE transpose pass.
  * Column tiling: each slab's two 512-col groups run concurrently on
    PE array columns 0-63 / 64-127.
"""
import numpy as np
import ml_dtypes

import concourse.mybir as mybir
from concourse import bacc
from concourse.tile import TileContext
from concourse.bass_utils import run_bass_kernel_spmd

F32 = mybir.dt.float32
F16 = mybir.dt.float16
F8 = mybir.dt.float8e5
GE = mybir.AluOpType.is_ge

B = 65536
NCORES = 8
PER = B // NCORES          # 8192 rows per core
SLAB = 1024                # rows per slab (2 column-tiled groups of 512)
NSLAB = PER // SLAB        # 8
NPAIR = NSLAB // 2         # hi tiles cover slab pairs (4KB lines)
PAIRW = 2 * SLAB
GRP = 512
K = 784
KCS = [128, 128, 128, 128, 128, 128, 16]   # contraction chunks (6x128+16)
KOF = [0, 128, 256, 384, 512, 640, 768]
NKC = len(KCS)
NCLS = 10
NHID = 50
LOSC = 4096.0              # residual scale 2^12

EPS = 1e-3

_CACHE = {}


def _build():
    nc = bacc.Bacc("TRN2", target_bir_lowering=False, debug=False,
                   num_devices=NCORES)

    xhi = nc.dram_tensor("xhi", [K, PER], F16, kind="ExternalInput").ap()
    xlo = nc.dram_tensor("xlo", [K, PER], F8, kind="ExternalInput").ap()
    cb16 = nc.dram_tensor("cb16", [128, NKC * NHID + NHID + NCLS], F16,
                          kind="ExternalInput").ap()
    cb8 = nc.dram_tensor("cb8", [128, NKC * NHID], F8,
                         kind="ExternalInput").ap()
    # col 0 = T1, col 1 = T2eff (both replicated at partition offset 64)
    cb32 = nc.dram_tensor("cb32", [128, 2], F32, kind="ExternalInput").ap()
    out16 = nc.dram_tensor("out16", [NCLS, PER], F16,
                           kind="ExternalOutput").ap()

    with TileContext(nc) as tc:
        with (
            tc.tile_pool(name="consts", bufs=1) as cpool,
            tc.tile_pool(name="xin", bufs=1) as xpool,
            tc.tile_pool(name="mid", bufs=3) as mpool,
            tc.tile_pool(name="fin", bufs=2) as fpool,
            tc.tile_pool(name="psA", bufs=4, space="PSUM") as psA,
            tc.tile_pool(name="psB", bufs=2, space="PSUM") as psB,
            tc.tile_pool(name="psC", bufs=2, space="PSUM") as psC,
        ):
            cb16t = cpool.tile([128, NKC * NHID + NHID + NCLS], F16, tag="cb16")
            nc.scalar.dma_start(cb16t[:], cb16[:, :])
            cb8t = cpool.tile([128, NKC * NHID], F8, tag="cb8")
            nc.scalar.dma_start(cb8t[:], cb8[:, :])
            cb32t = cpool.tile([128, 2], F32, tag="cb32")
            nc.scalar.dma_start(cb32t[:], cb32[:, :])

            w1h = [cb16t[0:KCS[c], c * NHID:(c + 1) * NHID]
                   for c in range(NKC)]
            w1l = [cb8t[0:KCS[c], c * NHID:(c + 1) * NHID]
                   for c in range(NKC)]
            w2t = cb16t[0:NHID, NKC * NHID:NKC * NHID + NHID]
            w2t64 = cb16t[64:64 + NHID, NKC * NHID:NKC * NHID + NHID]
            w3t = cb16t[0:NHID, NKC * NHID + NHID:NKC * NHID + NHID + NCLS]
            w3t64 = cb16t[64:64 + NHID,
                          NKC * NHID + NHID:NKC * NHID + NHID + NCLS]
            t1t = cb32t[0:64 + NHID, 0:1]
            t2t = cb32t[0:64 + NHID, 1:2]

            rings = [nc.sync, nc.scalar, nc.gpsimd]
            rbytes = [0.0, 0.0, 0.0]
            # gpsimd SWDGE issue is ~1us/dma: count it as extra ring cost
            GP_PEN = 256 * 1024

            def pick(sz, gp_ok=True):
                cand = range(3) if gp_ok else range(2)
                r = min(cand, key=lambda i: rbytes[i])
                rbytes[r] += sz + (GP_PEN if r == 2 else 0)
                return rings[r]

            # hi slab-pair tiles for pair 0 first (PE starts immediately)
            hp = [[None] * NKC for _ in range(NPAIR)]

            def load_hi(p):
                b0 = p * PAIRW
                for c in range(NKC):
                    t_ = xpool.tile([KCS[c], PAIRW], F16, tag=f"xh{p}_{c}")
                    eng = pick(KCS[c] * PAIRW * 2)
                    eng.dma_start(t_[:], xhi[KOF[c]:KOF[c] + KCS[c],
                                             b0:b0 + PAIRW])
                    hp[p][c] = t_

            load_hi(0)
            # all lo tiles early, spread across the rings
            lot = []
            for c in range(NKC):
                t_ = xpool.tile([KCS[c], PER], F8, tag=f"xl{c}")
                eng = pick(KCS[c] * PER)
                eng.dma_start(t_[:], xlo[KOF[c]:KOF[c] + KCS[c], :])
                lot.append(t_)
            for p in range(1, NPAIR):
                load_hi(p)

            s1t = {}
            s2t = {}

            def stageA(s):
                ps1 = psA.tile([128, GRP], F32, tag="ps1")
                p, h = s // 2, (s % 2) * SLAB
                for c in range(NKC):
                    st = (c == 0)
                    xin = hp[p][c][:, h:h + SLAB]
                    nc.tensor.matmul(ps1[0:NHID, :], w1h[c], xin[:, 0:GRP],
                                     start=st, stop=False,
                                     skip_group_check=True)
                    nc.tensor.matmul(ps1[64:64 + NHID, :], w1h[c],
                                     xin[:, GRP:2 * GRP],
                                     start=st, stop=False,
                                     skip_group_check=True)
                b0 = s * SLAB
                for c in range(NKC):
                    sp = (c == NKC - 1)
                    nc.tensor.matmul(ps1[0:NHID, :], w1l[c],
                                     lot[c][:, b0:b0 + GRP],
                                     start=False, stop=sp,
                                     skip_group_check=True)
                    nc.tensor.matmul(ps1[64:64 + NHID, :], w1l[c],
                                     lot[c][:, b0 + GRP:b0 + SLAB],
                                     start=False, stop=sp,
                                     skip_group_check=True)
                s1 = mpool.tile([64 + NHID, GRP], F16, tag="s1",
                                name=f"s1_{s}")
                nc.vector.tensor_scalar(s1[:], ps1[0:64 + NHID, :],
                                        t1t, None, GE)
                s1t[s] = s1

            def stageB(p):
                ps2 = psB.tile([128, GRP], F32, tag="ps2")
                s1 = s1t[p]
                nc.tensor.matmul(ps2[0:NHID, :], w2t, s1[0:NHID, :],
                                 start=True, stop=True, skip_group_check=True)
                nc.tensor.matmul(ps2[64:64 + NHID, :], w2t64,
                                 s1[64:64 + NHID, :],
                                 start=True, stop=True, skip_group_check=True)
                s2 = mpool.tile([64 + NHID, GRP], F16, tag="s2",
                                name=f"s2_{p}")
                nc.vector.tensor_scalar(s2[:], ps2[0:64 + NHID, :],
                                        t2t, None, GE)
                s2t[p] = s2

            def stageC(p):
                ps3 = psC.tile([74, GRP], F32, tag="ps3")
                s2 = s2t[p]
                nc.tensor.matmul(ps3[0:NCLS, :], w3t, s2[0:NHID, :],
                                 start=True, stop=True, skip_group_check=True)
                nc.tensor.matmul(ps3[64:64 + NCLS, :], w3t64,
                                 s2[64:64 + NHID, :],
                                 start=True, stop=True, skip_group_check=True)
                lg = fpool.tile([64 + NCLS, GRP], F16, tag="lg",
                                name=f"lg_{p}")
                nc.vector.tensor_scalar_add(lg[0:NCLS, :], ps3[0:NCLS, :], 0.0)
                nc.vector.tensor_scalar_add(lg[64:64 + NCLS, :],
                                            ps3[64:64 + NCLS, :], 0.0)
                b0 = p * SLAB
                nc.gpsimd.dma_start(out16[:, b0:b0 + GRP], lg[0:NCLS, :])
                nc.gpsimd.dma_start(out16[:, b0 + GRP:b0 + SLAB],
                                    lg[64:64 + NCLS, :])

            for p in range(NSLAB):
                stageA(p)
                if p >= 1:
                    stageB(p - 1)
                if p >= 2:
                    stageC(p - 2)
            stageB(NSLAB - 1)
            stageC(NSLAB - 2)
            stageC(NSLAB - 1)

    nc.compile()
    return nc


def _prep_host(inputs, W1, W2, W3, g1, b1, m1, v1, g2, b2, m2, v2):
    x = np.ascontiguousarray(inputs.reshape(B, K).astype(np.float32,
                                                         copy=False))
    xhi = x.astype(np.float16)
    xlo = ((x - xhi.astype(np.float32)) * LOSC).astype(ml_dtypes.float8_e5m2)

    w1b = np.where(W1 >= 0, 1.0, -1.0).astype(np.float16)
    w2b = np.where(W2 >= 0, 1.0, -1.0).astype(np.float16)
    w3b = np.where(W3 >= 0, 1.0, -1.0).astype(np.float16)
    w1l = (w1b.astype(np.float32) / LOSC).astype(ml_dtypes.float8_e5m2)

    def thresh(g, b, m, v):
        a = g.astype(np.float64) / np.sqrt(v.astype(np.float64) + EPS)
        c = b.astype(np.float64) - a * m.astype(np.float64)
        t = -c / a
        return np.where(t > 0, t, -1e30).astype(np.float32)

    T1 = thresh(g1, b1, m1, v1)
    T2 = thresh(g2, b2, m2, v2)
    c2 = w2b.astype(np.float32).sum(axis=0)
    T2e = ((T2 + c2) / 2).astype(np.float32)

    cb16 = np.zeros((128, NKC * NHID + NHID + NCLS), dtype=np.float16)
    cb8 = np.zeros((128, NKC * NHID), dtype=ml_dtypes.float8_e5m2)
    for c in range(NKC):
        cb16[:KCS[c], c * NHID:(c + 1) * NHID] = w1b[KOF[c]:KOF[c] + KCS[c]]
        cb8[:KCS[c], c * NHID:(c + 1) * NHID] = w1l[KOF[c]:KOF[c] + KCS[c]]
    cb16[:NHID, NKC * NHID:NKC * NHID + NHID] = w2b
    cb16[:NHID, NKC * NHID + NHID:] = w3b
    cb16[64:64 + NHID, NKC * NHID:NKC * NHID + NHID] = w2b
    cb16[64:64 + NHID, NKC * NHID + NHID:] = w3b
    cb32 = np.zeros((128, 2), dtype=np.float32)
    cb32[:NHID, 0] = T1
    cb32[64:64 + NHID, 0] = T1
    cb32[:NHID, 1] = T2e
    cb32[64:64 + NHID, 1] = T2e
    shared = {"cb16": cb16, "cb8": cb8, "cb32": cb32}
    in_maps = []
    for c in range(NCORES):
        sl = slice(c * PER, (c + 1) * PER)
        m = dict(shared)
        m["xhi"] = np.ascontiguousarray(xhi[sl].T)
        m["xlo"] = np.ascontiguousarray(xlo[sl].T)
        in_maps.append(m)
    return in_maps


def kernel(**inputs):
    if "nc" not in _CACHE:
        _CACHE["nc"] = _build()
    nc = _CACHE["nc"]
    inputs = {k: np.asarray(v) for k, v in inputs.items()}
    in_maps = _prep_host(**inputs)
    res = run_bass_kernel_spmd(nc, in_maps, core_ids=list(range(NCORES)))
    # device logits' use {1,0} activations: true logits = 2*lg' - colsum(W3b)
    w3b = np.where(inputs["W3"] >= 0, 1.0, -1.0).astype(np.float32)
    c3 = w3b.sum(axis=0)
    lg = np.concatenate([r["out16"] for r in res.results], axis=1)
    lg = 2.0 * lg.T.astype(np.float32) - c3                  # [B, 10]
    e = np.exp(lg - lg.max(axis=1, keepdims=True))
    return (e / e.sum(axis=1, keepdims=True)).astype(np.float32)


# revision 10
# speedup vs baseline: 1.4191x; 1.1444x over previous
"""BNN MNIST MLP on 8 Trainium2 NeuronCores — pure data parallel, v2d.

Model (inference): x[B,784] -> relu(x @ sign(W1)) -> BN1 -> sign ->
@ sign(W2) relu BN2 sign -> @ sign(W3) -> softmax.

Design (vs the 118us v1 baseline):
  * 25% less HBM traffic: x ships as fp16 hi (2B) + fp8-e5m2 residual
    (1B, scaled 2^12; the matching stationary sign(W1)*2^-12 is exact in
    e5m2).  Validated rel_err 8.45e-3 vs the 2e-2 gate.
  * Input fully SBUF-resident; every load issued up-front, unthrottled.
  * DMA efficiency: per-engine throughput ~ bytes/(45ns + bytes/24GBps),
    so lines must be >=4KB: hi ships as slab-PAIR chunk tiles
    ([128, 2048] fp16 -> 4KB lines) and lo as full-row chunk tiles
    ([128, 8192] fp8 -> 8KB lines).  128-row chunks (6x128+16) keep all
    partitions loaded.
  * lo tiles are spread across all three DMA rings and emitted early so
    the last-consumed bytes are hi of the last slab pair, not lo: kills
    the 18us drain tail of the single-queue version.  ps1 gets 4 PSUM
    banks so early slabs hold accumulations open while lo streams in.
  * Strict ring/engine separation: sync + scalar + gpsimd rings carry
    loads only (plus gpsimd the tiny stores), DVE does all elementwise
    work (is_ge binarize in {1,0} encoding, exact via threshold folding
    t2' = (t2+colsum(W2b))/2 and host-side logits = 2*lg' - colsum(W3b)),
    PE only matmuls.  ScalarE runs nothing per-slab.
  * Layer 3 emits class-major logits ([10, B], half-integers, exact in
    fp16); softmax + transpose on host.  No PE transpose pass.
  * Column tiling: each slab's two 512-col groups run concurrently on
    PE array columns 0-63 / 64-127.
"""
import numpy as np
import ml_dtypes

import concourse.mybir as mybir
from concourse import bacc
from concourse.tile import TileContext
from concourse.bass_utils import run_bass_kernel_spmd

F32 = mybir.dt.float32
F16 = mybir.dt.float16
F8 = mybir.dt.float8e5
GE = mybir.AluOpType.is_ge

B = 65536
NCORES = 8
PER = B // NCORES          # 8192 rows per core
SLAB = 1024                # rows per slab (2 column-tiled groups of 512)
NSLAB = PER // SLAB        # 8
NPAIR = NSLAB // 2         # hi tiles cover slab pairs (4KB lines)
PAIRW = 2 * SLAB
GRP = 512
K = 784
KCS = [128, 128, 128, 128, 128, 128, 16]   # contraction chunks (6x128+16)
KOF = [0, 128, 256, 384, 512, 640, 768]
NKC = len(KCS)
NCLS = 10
NHID = 50
LOSC = 4096.0              # residual scale 2^12

EPS = 1e-3

_CACHE = {}


def _build():
    nc = bacc.Bacc("TRN2", target_bir_lowering=False, debug=False,
                   num_devices=NCORES)

    xhi = nc.dram_tensor("xhi", [K, PER], F16, kind="ExternalInput").ap()
    xlo = nc.dram_tensor("xlo", [K, PER], F8, kind="ExternalInput").ap()
    cb16 = nc.dram_tensor("cb16", [128, NKC * NHID + NHID + NCLS], F16,
                          kind="ExternalInput").ap()
    cb8 = nc.dram_tensor("cb8", [128, NKC * NHID], F8,
                         kind="ExternalInput").ap()
    # col 0 = T1, col 1 = T2eff (both replicated at partition offset 64)
    cb32 = nc.dram_tensor("cb32", [128, 2], F32, kind="ExternalInput").ap()
    out16 = nc.dram_tensor("out16", [NCLS, PER], F16,
                           kind="ExternalOutput").ap()

    with TileContext(nc) as tc:
        with (
            tc.tile_pool(name="consts", bufs=1) as cpool,
            tc.tile_pool(name="xin", bufs=1) as xpool,
            tc.tile_pool(name="mid", bufs=3) as mpool,
            tc.tile_pool(name="fin", bufs=2) as fpool,
            tc.tile_pool(name="psA", bufs=4, space="PSUM") as psA,
            tc.tile_pool(name="psB", bufs=2, space="PSUM") as psB,
            tc.tile_pool(name="psC", bufs=2, space="PSUM") as psC,
        ):
            cb16t = cpool.tile([128, NKC * NHID + NHID + NCLS], F16, tag="cb16")
            nc.scalar.dma_start(cb16t[:], cb16[:, :])
            cb8t = cpool.tile([128, NKC * NHID], F8, tag="cb8")
            nc.scalar.dma_start(cb8t[:], cb8[:, :])
            cb32t = cpool.tile([128, 2], F32, tag="cb32")
            nc.scalar.dma_start(cb32t[:], cb32[:, :])

            w1h = [cb16t[0:KCS[c], c * NHID:(c + 1) * NHID]
                   for c in range(NKC)]
            w1l = [cb8t[0:KCS[c], c * NHID:(c + 1) * NHID]
                   for c in range(NKC)]
            w2t = cb16t[0:NHID, NKC * NHID:NKC * NHID + NHID]
            w2t64 = cb16t[64:64 + NHID, NKC * NHID:NKC * NHID + NHID]
            w3t = cb16t[0:NHID, NKC * NHID + NHID:NKC * NHID + NHID + NCLS]
            w3t64 = cb16t[64:64 + NHID,
                          NKC * NHID + NHID:NKC * NHID + NHID + NCLS]
            t1t = cb32t[0:64 + NHID, 0:1]
            t2t = cb32t[0:64 + NHID, 1:2]

            # loads only ever on the two HWDGE rings (the gpsimd SWDGE
            # queue tops out at ~62GB/s and straggles 20us past them)
            rings = [nc.sync, nc.scalar]
            rbytes = [0.0, 0.0]

            def pick(sz):
                r = min(range(2), key=lambda i: rbytes[i])
                rbytes[r] += sz
                return rings[r]

            # hi slab-pair tiles for pair 0 first (PE starts immediately)
            hp = [[None] * NKC for _ in range(NPAIR)]

            def load_hi(p):
                b0 = p * PAIRW
                for c in range(NKC):
                    t_ = xpool.tile([KCS[c], PAIRW], F16, tag=f"xh{p}_{c}")
                    eng = pick(KCS[c] * PAIRW * 2)
                    eng.dma_start(t_[:], xhi[KOF[c]:KOF[c] + KCS[c],
                                             b0:b0 + PAIRW])
                    hp[p][c] = t_

            load_hi(0)
            # all lo tiles early, spread across the rings
            lot = []
            for c in range(NKC):
                t_ = xpool.tile([KCS[c], PER], F8, tag=f"xl{c}")
                eng = pick(KCS[c] * PER)
                eng.dma_start(t_[:], xlo[KOF[c]:KOF[c] + KCS[c], :])
                lot.append(t_)
            for p in range(1, NPAIR):
                load_hi(p)

            s1t = {}
            s2t = {}

            def stageA(s):
                ps1 = psA.tile([128, GRP], F32, tag="ps1")
                p, h = s // 2, (s % 2) * SLAB
                for c in range(NKC):
                    st = (c == 0)
                    xin = hp[p][c][:, h:h + SLAB]
                    nc.tensor.matmul(ps1[0:NHID, :], w1h[c], xin[:, 0:GRP],
                                     start=st, stop=False,
                                     skip_group_check=True)
                    nc.tensor.matmul(ps1[64:64 + NHID, :], w1h[c],
                                     xin[:, GRP:2 * GRP],
                                     start=st, stop=False,
                                     skip_group_check=True)
                b0 = s * SLAB
                for c in range(NKC):
                    sp = (c == NKC - 1)
                    nc.tensor.matmul(ps1[0:NHID, :], w1l[c],
                                     lot[c][:, b0:b0 + GRP],
                                     start=False, stop=sp,
                                     skip_group_check=True)
                    nc.tensor.matmul(ps1[64:64 + NHID, :], w1l[c],
                                     lot[c][:, b0 + GRP:b0 + SLAB],
                                     start=False, stop=sp,
                                     skip_group_check=True)
                s1 = mpool.tile([64 + NHID, GRP], F16, tag="s1",
                                name=f"s1_{s}")
                nc.vector.tensor_scalar(s1[:], ps1[0:64 + NHID, :],
                                        t1t, None, GE)
                s1t[s] = s1

            def stageB(p):
                ps2 = psB.tile([128, GRP], F32, tag="ps2")
                s1 = s1t[p]
                nc.tensor.matmul(ps2[0:NHID, :], w2t, s1[0:NHID, :],
                                 start=True, stop=True, skip_group_check=True)
                nc.tensor.matmul(ps2[64:64 + NHID, :], w2t64,
                                 s1[64:64 + NHID, :],
                                 start=True, stop=True, skip_group_check=True)
                s2 = mpool.tile([64 + NHID, GRP], F16, tag="s2",
                                name=f"s2_{p}")
                nc.vector.tensor_scalar(s2[:], ps2[0:64 + NHID, :],
                                        t2t, None, GE)
                s2t[p] = s2

            def stageC(p):
                ps3 = psC.tile([74, GRP], F32, tag="ps3")
                s2 = s2t[p]
                nc.tensor.matmul(ps3[0:NCLS, :], w3t, s2[0:NHID, :],
                                 start=True, stop=True, skip_group_check=True)
                nc.tensor.matmul(ps3[64:64 + NCLS, :], w3t64,
                                 s2[64:64 + NHID, :],
                                 start=True, stop=True, skip_group_check=True)
                lg = fpool.tile([64 + NCLS, GRP], F16, tag="lg",
                                name=f"lg_{p}")
                nc.vector.tensor_scalar_add(lg[0:NCLS, :], ps3[0:NCLS, :], 0.0)
                nc.vector.tensor_scalar_add(lg[64:64 + NCLS, :],
                                            ps3[64:64 + NCLS, :], 0.0)
                b0 = p * SLAB
                nc.sync.dma_start(out16[:, b0:b0 + GRP], lg[0:NCLS, :])
                nc.scalar.dma_start(out16[:, b0 + GRP:b0 + SLAB],
                                    lg[64:64 + NCLS, :])

            for p in range(NSLAB):
                stageA(p)
                if p >= 1:
                    stageB(p - 1)
                if p >= 2:
                    stageC(p - 2)
            stageB(NSLAB - 1)
            stageC(NSLAB - 2)
            stageC(NSLAB - 1)

    nc.compile()
    return nc


def _prep_host(inputs, W1, W2, W3, g1, b1, m1, v1, g2, b2, m2, v2):
    x = np.ascontiguousarray(inputs.reshape(B, K).astype(np.float32,
                                                         copy=False))
    xhi = x.astype(np.float16)
    xlo = ((x - xhi.astype(np.float32)) * LOSC).astype(ml_dtypes.float8_e5m2)

    w1b = np.where(W1 >= 0, 1.0, -1.0).astype(np.float16)
    w2b = np.where(W2 >= 0, 1.0, -1.0).astype(np.float16)
    w3b = np.where(W3 >= 0, 1.0, -1.0).astype(np.float16)
    w1l = (w1b.astype(np.float32) / LOSC).astype(ml_dtypes.float8_e5m2)

    def thresh(g, b, m, v):
        a = g.astype(np.float64) / np.sqrt(v.astype(np.float64) + EPS)
        c = b.astype(np.float64) - a * m.astype(np.float64)
        t = -c / a
        return np.where(t > 0, t, -1e30).astype(np.float32)

    T1 = thresh(g1, b1, m1, v1)
    T2 = thresh(g2, b2, m2, v2)
    c2 = w2b.astype(np.float32).sum(axis=0)
    T2e = ((T2 + c2) / 2).astype(np.float32)

    cb16 = np.zeros((128, NKC * NHID + NHID + NCLS), dtype=np.float16)
    cb8 = np.zeros((128, NKC * NHID), dtype=ml_dtypes.float8_e5m2)
    for c in range(NKC):
        cb16[:KCS[c], c * NHID:(c + 1) * NHID] = w1b[KOF[c]:KOF[c] + KCS[c]]
        cb8[:KCS[c], c * NHID:(c + 1) * NHID] = w1l[KOF[c]:KOF[c] + KCS[c]]
    cb16[:NHID, NKC * NHID:NKC * NHID + NHID] = w2b
    cb16[:NHID, NKC * NHID + NHID:] = w3b
    cb16[64:64 + NHID, NKC * NHID:NKC * NHID + NHID] = w2b
    cb16[64:64 + NHID, NKC * NHID + NHID:] = w3b
    cb32 = np.zeros((128, 2), dtype=np.float32)
    cb32[:NHID, 0] = T1
    cb32[64:64 + NHID, 0] = T1
    cb32[:NHID, 1] = T2e
    cb32[64:64 + NHID, 1] = T2e
    shared = {"cb16": cb16, "cb8": cb8, "cb32": cb32}
    in_maps = []
    for c in range(NCORES):
        sl = slice(c * PER, (c + 1) * PER)
        m = dict(shared)
        m["xhi"] = np.ascontiguousarray(xhi[sl].T)
        m["xlo"] = np.ascontiguousarray(xlo[sl].T)
        in_maps.append(m)
    return in_maps


def kernel(**inputs):
    if "nc" not in _CACHE:
        _CACHE["nc"] = _build()
    nc = _CACHE["nc"]
    inputs = {k: np.asarray(v) for k, v in inputs.items()}
    in_maps = _prep_host(**inputs)
    res = run_bass_kernel_spmd(nc, in_maps, core_ids=list(range(NCORES)))
    # device logits' use {1,0} activations: true logits = 2*lg' - colsum(W3b)
    w3b = np.where(inputs["W3"] >= 0, 1.0, -1.0).astype(np.float32)
    c3 = w3b.sum(axis=0)
    lg = np.concatenate([r["out16"] for r in res.results], axis=1)
    lg = 2.0 * lg.T.astype(np.float32) - c3                  # [B, 10]
    e = np.exp(lg - lg.max(axis=1, keepdims=True))
    return (e / e.sum(axis=1, keepdims=True)).astype(np.float32)
